# revision 35
# baseline (speedup 1.0000x reference)
"""Trainium2 Bass kernel for nn_CaptionModel (GRU caption decoder).

Model: h0 = feat; x0 = embed[<SOS>]; 200 GRU steps where the *output hidden
state is fed back as the next input* (x_t = h_t for t >= 1), then a linear
projection of every hidden state to vocab logits, output [B, V, T].

Because x_t == h_t for t >= 1, the two GRU matmuls fuse into one:
  G = h @ Wc.T + bc  with  Wc = [w_ih_r+w_hh_r; w_ih_z+w_hh_z; w_ih_n; w_hh_n]
  r = sig(G0), z = sig(G1), n = tanh(G2 + r*G3), h' = (1-z)*n + z*h
Step 0 folds x0 through w_ih into a modified bias (gi0) and uses w_hh only.

Sharding: pure data parallelism, batch 256 -> 32 per core on 8 cores,
weights replicated. Per-core layout: batch on PSUM partitions (M=32),
gates on the free dim, contraction H=512 as 4 k-chunks of 128 with the
transposed hidden state as the (tiny) stationary operand and the f32r
weights streamed as the moving operand (1 cyc/col at N>=512). Biases are
injected via K=1 matmuls of a ones-row. The new h is transposed back with
4 PE-transposes per step straight into a history buffer that serves as
(a) the next step's stationary operand and (b) the projection's rhs.
"""

import os
from contextlib import ExitStack

import numpy as np

import concourse.bass as bass
import concourse.tile as tile
from concourse import bacc, mybir
from concourse.bass_utils import run_bass_kernel_spmd

B, H, VOCAB = 256, 512, 100
STEPS = int(os.environ.get("KERNEL_STEPS", "200"))
NCORES = 8
BD = B // NCORES  # 32
KC = H // 128  # 4 k-chunks
G4 = 4 * H  # 2048 fused gate width
PB = 2  # batch rows per projection chunk
F32 = mybir.dt.float32
F32R = mybir.dt.float32r


def _build(steps: int):
    nc = bacc.Bacc("TRN2", target_bir_lowering=False, debug=False,
                   num_devices=NCORES)

    feat_d = nc.dram_tensor("feat", [BD, H], F32, kind="ExternalInput").ap()
    wct_d = nc.dram_tensor("wct", [KC, 128, G4], F32R, kind="ExternalInput").ap()
    wc0t_d = nc.dram_tensor("wc0t", [KC, 128, 3 * H], F32R, kind="ExternalInput").ap()
    bct_d = nc.dram_tensor("bct", [1, G4], F32R, kind="ExternalInput").ap()
    bc0t_d = nc.dram_tensor("bc0t", [1, G4], F32R, kind="ExternalInput").ap()
    projt_d = nc.dram_tensor("projt", [KC, 128, VOCAB], F32R, kind="ExternalInput").ap()
    projb_d = nc.dram_tensor("projb", [VOCAB, 1], F32, kind="ExternalInput").ap()
    ident_d = nc.dram_tensor("ident", [32, 32], F32, kind="ExternalInput").ap()
    ones_d = nc.dram_tensor("ones", [1, BD], F32R, kind="ExternalInput").ap()
    out_d = nc.dram_tensor("out", [BD, VOCAB, steps], F32, kind="ExternalOutput").ap()

    SIG = mybir.ActivationFunctionType.Sigmoid
    TANH = mybir.ActivationFunctionType.Tanh

    with tile.TileContext(nc) as tc, ExitStack() as ctx:
        singles = ctx.enter_context(tc.tile_pool(name="singles", bufs=1))
        hpool = ctx.enter_context(tc.tile_pool(name="h", bufs=2))
        work = ctx.enter_context(tc.tile_pool(name="work", bufs=1))

        # --- resident constants / weights ---
        ident_s = singles.tile([32, 32], F32)
        nc.sync.dma_start(out=ident_s, in_=ident_d)
        ones_s = singles.tile([1, BD], F32R)
        nc.sync.dma_start(out=ones_s, in_=ones_d)
        bc0t_s = singles.tile([1, G4], F32R)
        nc.sync.dma_start(out=bc0t_s, in_=bc0t_d)
        bct_s = singles.tile([1, G4], F32R)
        nc.sync.dma_start(out=bct_s, in_=bct_d)
        wc0t_s = singles.tile([128, KC, 3 * H], F32R)
        nc.sync.dma_start(out=wc0t_s, in_=wc0t_d.rearrange("c p n -> p c n"))
        wct_s = singles.tile([128, KC, G4], F32R)
        nc.sync.dma_start(out=wct_s, in_=wct_d.rearrange("c p n -> p c n"))
        projt_s = singles.tile([128, KC, VOCAB], F32R)
        nc.sync.dma_start(out=projt_s, in_=projt_d.rearrange("c p n -> p c n"))
        projb_s = singles.tile([VOCAB, 1], F32)
        nc.sync.dma_start(out=projb_s, in_=projb_d)

        # history of transposed hidden states: hist[c][p, b, t] = h_t[b, c*128+p]
        hist = [singles.tile([128, BD, steps], F32R, tag=f"hist{c}", name=f"hist{c}")
                for c in range(KC)]
        hT0_s = singles.tile([128, KC, BD], F32R)

        with tc.tile_pool(name="gpsum", bufs=1, space="PSUM") as gpool, \
             tc.tile_pool(name="tpsum", bufs=2, space="PSUM") as tpool:

            # --- h0 = feat; build transposed h0 ---
            h_first = hpool.tile([BD, H], F32, tag="h")
            nc.sync.dma_start(out=h_first, in_=feat_d)
            for c in range(KC):
                tp = tpool.tile([128, BD], F32, tag="tp")
                nc.tensor.transpose(tp, h_first[:, c * 128:(c + 1) * 128], ident_s)
                nc.scalar.copy(out=hT0_s[:, c, :], in_=tp)

            h_prev = h_first

            def emit_step(t, h_mid):
                """Emit step t's matmuls (interleaved with the transposes of
                h_mid = h_{t-1} into hist[.., t-1]) and the gate math,
                returning h_t. For t==0, h_mid is None (hT0 pre-built)."""
                bias_s = bc0t_s if t == 0 else bct_s

                def lhsT(c):
                    return hT0_s[:, c, :] if t == 0 else hist[c][:, :, t - 1]

                def wslice(g, lo, hi):
                    if t == 0:
                        col0 = {0: 0, 1: 512, 3: 1024}[g]
                        return wc0t_s, col0 + lo, col0 + hi
                    return wct_s, g * 512 + lo, g * 512 + hi

                # six psum accumulators, one bank each
                r_ps = gpool.tile([BD, 512], F32, tag="r_ps")
                z_ps = gpool.tile([BD, 512], F32, tag="z_ps")
                hnA_ps = gpool.tile([BD, 256], F32, tag="hnA_ps")
                hnB_ps = gpool.tile([BD, 256], F32, tag="hnB_ps")
                inA_ps = gpool.tile([BD, 256], F32, tag="inA_ps")
                inB_ps = gpool.tile([BD, 256], F32, tag="inB_ps")

                def tc(c):
                    if h_mid is None:
                        return
                    tp = tpool.tile([128, BD], F32, tag="tp")
                    nc.tensor.transpose(
                        tp, h_mid[:, c * 128:(c + 1) * 128], ident_s)
                    nc.scalar.copy(out=hist[c][:, :, t - 1], in_=tp)

                def kmm(ps, g, lo, hi, c):
                    w_ap, wlo, whi = wslice(g, lo, hi)
                    nc.tensor.matmul(ps, lhsT(c), w_ap[:, c, wlo:whi],
                                     start=False, stop=(c == KC - 1))

                def bias_mm(ps, g, lo, hi, stop=False):
                    nc.tensor.matmul(ps, ones_s,
                                     bias_s[:, g * 512 + lo:g * 512 + hi],
                                     start=True, stop=stop)

                in_bias_only = (t == 0)
                bias_mm(r_ps, 0, 0, 512)
                bias_mm(z_ps, 1, 0, 512)
                # interleave: transpose chunk c of h_{t-1}, then the k=c
                # matmuls of r and z that consume it
                for c in range(KC):
                    tc(c)
                    kmm(r_ps, 0, 0, 512, c)
                    kmm(z_ps, 1, 0, 512, c)
                for hn_ps, in_ps, lo, hi in ((hnA_ps, inA_ps, 0, 256),
                                             (hnB_ps, inB_ps, 256, 512)):
                    bias_mm(in_ps, 2, lo, hi, stop=in_bias_only)
                    if not in_bias_only:
                        for c in range(KC):
                            kmm(in_ps, 2, lo, hi, c)
                    bias_mm(hn_ps, 3, lo, hi)
                    for c in range(KC):
                        kmm(hn_ps, 3, lo, hi, c)

                r_s = work.tile([BD, H], F32, tag="r")
                nc.scalar.activation(r_s[:, 0:256], r_ps[:, 0:256], SIG)
                nc.scalar.activation(r_s[:, 256:512], r_ps[:, 256:512], SIG)
                z_s = work.tile([BD, H], F32, tag="z")
                nc.scalar.activation(z_s, z_ps, SIG)
                z1m_s = work.tile([BD, H], F32, tag="z1m")
                nc.scalar.activation(z1m_s, z_ps, SIG, scale=-1.0)
                u_s = work.tile([BD, H], F32, tag="u")
                nc.gpsimd.tensor_mul(u_s, z_s, h_prev)

                a_s = work.tile([BD, H], F32, tag="a")
                b_s = work.tile([BD, H], F32, tag="b")
                n_s = work.tile([BD, H], F32, tag="n")
                h_new = hpool.tile([BD, H], F32, tag="h")

                e_s = work.tile([BD, H], F32, tag="e")

                def npath_chain(hn_ps, in_ps, lo, skip_a=False):
                    qs = slice(lo, lo + 256)
                    if not skip_a:
                        nc.vector.tensor_mul(a_s[:, qs], r_s[:, qs], hn_ps)
                    nc.vector.tensor_add(b_s[:, qs], a_s[:, qs], in_ps)
                    nc.scalar.activation(n_s[:, qs], b_s[:, qs], TANH)
                    nc.vector.tensor_mul(e_s[:, qs], z1m_s[:, qs], n_s[:, qs])
                    nc.vector.tensor_add(h_new[:, qs], u_s[:, qs], e_s[:, qs])

                # fast-path the first 128-col quarter: it alone gates
                # T0 -> c0 -> next step's k0 matmuls
                for q in (0, 1):
                    qs = slice(q * 128, (q + 1) * 128)
                    nc.vector.tensor_mul(a_s[:, qs], r_s[:, qs], hnA_ps[:, qs])
                    nc.vector.tensor_add(b_s[:, qs], a_s[:, qs], inA_ps[:, qs])
                    nc.scalar.activation(n_s[:, qs], b_s[:, qs], TANH)
                    nc.vector.tensor_mul(e_s[:, qs], z1m_s[:, qs], n_s[:, qs])
                    nc.vector.tensor_add(h_new[:, qs], u_s[:, qs], e_s[:, qs])
                for q in (2, 3):
                    qs = slice(q * 128, (q + 1) * 128)
                    ps_q = slice((q - 2) * 128, (q - 1) * 128)
                    nc.vector.tensor_mul(a_s[:, qs], r_s[:, qs], hnB_ps[:, ps_q])
                    nc.vector.tensor_add(b_s[:, qs], a_s[:, qs], inB_ps[:, ps_q])
                    nc.scalar.activation(n_s[:, qs], b_s[:, qs], TANH)
                    nc.vector.tensor_mul(e_s[:, qs], z1m_s[:, qs], n_s[:, qs])
                    nc.vector.tensor_add(h_new[:, qs], u_s[:, qs], e_s[:, qs])
                return h_new

            reps = int(os.environ.get("KERNEL_REPS", "1"))
            for rep in range(reps):
                for t in range(steps):
                    if rep == 0 and t == 0:
                        h_new = emit_step(0, None)
                    elif t == 0:
                        continue  # bench-only replication skips step 0
                    else:
                        h_new = emit_step(t, h_prev)
                    h_prev = h_new

            # final state still needs transposing into hist[.., steps-1]
            for c in range(KC):
                tp = tpool.tile([128, BD], F32, tag="tp")
                nc.tensor.transpose(tp, h_prev[:, c * 128:(c + 1) * 128], ident_s)
                nc.scalar.copy(out=hist[c][:, :, steps - 1], in_=tp)

        # --- projection: logits[v, b, t] = proj_w @ h + proj_b ---
        with tc.tile_pool(name="ppsum", bufs=2, space="PSUM") as ppool, \
             tc.tile_pool(name="stage", bufs=2) as spool:
            NW = PB * steps
            for j in range(BD // PB):
                P = ppool.tile([VOCAB, NW], F32, tag="P")
                for c in range(KC):
                    rhs = hist[c][:, j * PB:(j + 1) * PB, :].rearrange(
                        "p b t -> p (b t)")
                    nc.tensor.matmul(P, projt_s[:, c, :], rhs,
                                     start=(c == 0), stop=(c == KC - 1))
                stage = spool.tile([VOCAB, NW], F32, tag="stage")
                nc.vector.tensor_scalar_add(stage, P, projb_s)
                nc.sync.dma_start(
                    out=out_d[j * PB:(j + 1) * PB].rearrange("b v t -> v b t"),
                    in_=stage.rearrange("p (b t) -> p b t", b=PB))

    nc.compile()
    return nc


_CACHE = {}


def _get_nc(steps: int):
    if steps not in _CACHE:
        _CACHE[steps] = _build(steps)
    return _CACHE[steps]


def _prep_inputs(feat, embed_table, w_ih, w_hh, b_ih, b_hh, proj_w, proj_b):
    f32 = np.float32
    w_ih = np.asarray(w_ih, f32)
    w_hh = np.asarray(w_hh, f32)
    b_ih = np.asarray(b_ih, f32)
    b_hh = np.asarray(b_hh, f32)
    Wc = np.concatenate([w_ih[:H] + w_hh[:H], w_ih[H:2 * H] + w_hh[H:2 * H],
                         w_ih[2 * H:], w_hh[2 * H:]], 0)  # [4H, H]
    bc = np.concatenate([b_ih[:H] + b_hh[:H], b_ih[H:2 * H] + b_hh[H:2 * H],
                         b_ih[2 * H:], b_hh[2 * H:]], 0)  # [4H]
    x0 = np.asarray(embed_table, f32)[0]
    gi0 = w_ih @ x0 + b_ih
    bc0 = np.concatenate([gi0[:H] + b_hh[:H], gi0[H:2 * H] + b_hh[H:2 * H],
                          gi0[2 * H:], b_hh[2 * H:]], 0)
    Wc0 = np.concatenate([w_hh[:H], w_hh[H:2 * H], w_hh[2 * H:]], 0)  # [3H, H]

    common = {
        "wct": np.ascontiguousarray(Wc.T.reshape(KC, 128, G4)),
        "wc0t": np.ascontiguousarray(Wc0.T.reshape(KC, 128, 3 * H)),
        "bct": bc.reshape(1, G4),
        "bc0t": bc0.reshape(1, G4),
        "projt": np.ascontiguousarray(
            np.asarray(proj_w, f32).T.reshape(KC, 128, VOCAB)),
        "projb": np.asarray(proj_b, f32).reshape(VOCAB, 1),
        "ident": np.eye(32, dtype=f32),
        "ones": np.ones((1, BD), f32),
    }
    feat = np.asarray(feat, f32)
    return [dict(common, feat=np.ascontiguousarray(feat[i * BD:(i + 1) * BD]))
            for i in range(NCORES)]


def kernel(feat, embed_table, w_ih, w_hh, b_ih, b_hh, proj_w, proj_b,
           _trace=False):
    nc = _get_nc(STEPS)
    in_maps = _prep_inputs(feat, embed_table, w_ih, w_hh, b_ih, b_hh,
                           proj_w, proj_b)
    res = run_bass_kernel_spmd(nc, in_maps, list(range(NCORES)), trace=_trace)
    out = np.concatenate([res.results[i]["out"] for i in range(NCORES)], 0)
    if _trace:
        kernel.last_exec_time_ns = res.exec_time_ns
        kernel.last_results = res
    return out
